# revision 1
# baseline (speedup 1.0000x reference)
"""CDiceLoss Trainium2 kernel, v2.

Shards B*HW over 8 cores (core = one (batch, half-of-HW) slice). Host packs
two fp8(e4m3) slabs per core:

  xt  [128, 171*121]  pre-transposed gram blocks: block t is [128 k-pos, 121]
                      with cols = 120 (group,channel) x-values + ones col.
                      PE matmuls block.T @ block accumulating in PSUM ->
                      G (6 diagonal 20x20 blocks) and sum_x (ones column).
  z   [128, 16384]    |x + y - 1| for the 16 known channels, densely packed
                      as rows (s, c) = s*16+c, s in [0,8). Feeds:
                        DVE reduce-add  -> sum|z|   (dice numerator)
                        ACT Ln + accum  -> sum ln|z| (= BCE sum)

sum_y is an exact host-side integer count; the host combines the tiny
per-core stats into (loss, loss1, loss2, loss3).
"""

import os
from contextlib import ExitStack

import numpy as np
import ml_dtypes

import concourse.bass as bass
import concourse.bacc as bacc
import concourse.tile as tile
from concourse import mybir
from concourse.bass_utils import run_bass_kernel_spmd

# ---------------- problem geometry (hardcoded) ----------------
B, C, H, W = 4, 20, 512, 512
HW = H * W                  # 262144
KNOWN = 16
SMOOTH = 1.0
NCORES = 8
HWH = HW // 2               # 131072 positions per core
NG = 6                      # channel-groups per gram block
L = 21888                   # padded per-group length: 6*21888 = HWH + 256
NT = L // 128               # 171 gram blocks
BCOL = NG * C + 1           # 121 cols per block (120 x-rows + ones)
XTW = NT * BCOL             # 20691
NXC = 9                     # xt chunks
XBPC = NT // NXC            # 19 blocks per chunk
ZW = KNOWN * HWH // 128     # 16384
NZC = 8                     # z chunks
ZCW = ZW // NZC             # 2048

FP32 = mybir.dt.float32
BF16 = mybir.dt.bfloat16
F8 = mybir.dt.float8e4
NPF8 = ml_dtypes.float8_e4m3
OP = mybir.AluOpType
AF = mybir.ActivationFunctionType
AX = mybir.AxisListType

_CACHE = {}


def _build():
    if "nc" in _CACHE:
        return _CACHE["nc"]

    nc = bacc.Bacc(
        "TRN2", target_bir_lowering=False, debug=False, num_devices=NCORES
    )

    xt_d = nc.dram_tensor("xt", [128, XTW], F8, kind="ExternalInput").ap()
    z_d = nc.dram_tensor("z", [128, ZW], F8, kind="ExternalInput").ap()

    g_d = nc.dram_tensor("g_out", [BCOL, BCOL], FP32, kind="ExternalOutput").ap()
    st_d = nc.dram_tensor("st_out", [128, 2 * NZC], FP32, kind="ExternalOutput").ap()

    with tile.TileContext(nc) as tc, ExitStack() as ctx:
        sing = ctx.enter_context(tc.tile_pool(name="sing", bufs=1))
        xpool = ctx.enter_context(tc.tile_pool(name="xpool", bufs=NXC))
        zpool = ctx.enter_context(tc.tile_pool(name="zpool", bufs=NZC))
        lnpool = ctx.enter_context(tc.tile_pool(name="lnpool", bufs=2))
        gp_pool = ctx.enter_context(tc.tile_pool(name="gp", bufs=1, space="PSUM"))
        wp_pool = ctx.enter_context(tc.tile_pool(name="wp", bufs=1, space="PSUM"))

        # stats: cols [0, NZC) = sum|z| per chunk, [NZC, 2*NZC) = sum ln|z|
        stats = sing.tile([128, 2 * NZC], FP32)

        g_ps = gp_pool.tile([128, BCOL], FP32)

        # Ln ACT table load at t=0, off the first z chunk's critical path.
        tdum = sing.tile([1, 8], BF16)
        nc.vector.memset(tdum[:, :], 0.5)
        tdum2 = sing.tile([1, 8], BF16)
        nc.scalar.activation(out=tdum2[:, :], in_=tdum[:, :], func=AF.Ln)

        # PE warmup: finite weights + a dummy-matmul burst during the first
        # DMAs trips the PE HAM clock-gate to 8/8 before real grams arrive.
        warm = sing.tile([128, 128], BF16)
        nc.vector.memset(warm[:, :], 0.5)
        wps = wp_pool.tile([128, 128], FP32)
        for _ in range(24):
            nc.tensor.matmul(
                out=wps[:, :], lhsT=warm[:, :], rhs=warm[:, :],
                start=True, stop=True, skip_group_check=True,
            )

        xtiles = []
        ztiles = []
        # Issue all input DMAs up-front, interleaved z/xt so every engine's
        # stream starts early; SBUF holds both slabs entirely (no recycling).
        for i in range(NXC):
            if i < NZC:
                zt = zpool.tile([128, ZCW], F8, tag="zt")
                nc.sync.dma_start(
                    out=zt[:, :], in_=z_d[:, i * ZCW : (i + 1) * ZCW]
                )
                ztiles.append(zt)
            xt = xpool.tile([128, XBPC * BCOL], F8, tag="xt")
            nc.sync.dma_start(
                out=xt[:, :],
                in_=xt_d[:, i * XBPC * BCOL : (i + 1) * XBPC * BCOL],
            )
            xtiles.append(xt)

        mm = 0
        for i in range(NXC):
            if i < NZC:
                zt = ztiles[i]
                # DVE: sum|z| per row for this chunk
                nc.vector.tensor_reduce(
                    out=stats[:, i : i + 1], in_=zt[:, :],
                    axis=AX.X, op=OP.add,
                )
                # ACT: sum ln|z| per row (accumulator), ln output discarded
                lnt = lnpool.tile([128, ZCW], BF16, tag="lnt")
                nc.scalar.activation(
                    out=lnt[:, :], in_=zt[:, :], func=AF.Ln,
                    accum_out=stats[:, NZC + i : NZC + i + 1],
                )
            xt = xtiles[i]
            for j in range(XBPC):
                sl = slice(j * BCOL, (j + 1) * BCOL)
                mm += 1
                nc.tensor.matmul(
                    out=g_ps[0:BCOL, :],
                    lhsT=xt[:, sl],
                    rhs=xt[:, sl],
                    start=(mm == 1),
                    stop=(mm == NT),
                    skip_group_check=True,
                )
        assert mm == NT, mm

        g_sb = sing.tile([128, BCOL], FP32)
        nc.vector.tensor_copy(out=g_sb[0:BCOL, :], in_=g_ps[0:BCOL, :])
        nc.sync.dma_start(out=g_d, in_=g_sb[0:BCOL, :])
        nc.sync.dma_start(out=st_d, in_=stats[:, :])

    nc.compile()
    _CACHE["nc"] = nc
    return nc


def _pack_core(Xc, Yc):
    """Xc [20, HWH] f32, Yc [16, HWH] f32 -> (xt, z) fp8 slabs."""
    Zc = np.abs(Xc[:KNOWN] + Yc - 1.0)
    # rows r = s*16 + c, s in [0,8): z[c, s*16384 + j]
    z = Zc.reshape(KNOWN, 8, HWH // 8).transpose(1, 0, 2).reshape(128, ZW)
    z8 = np.ascontiguousarray(z.astype(NPF8))

    xp = np.zeros((C, NG * L), np.float32)
    xp[:, :HWH] = Xc
    arr = xp.reshape(C, NG, NT, 128).transpose(3, 2, 1, 0)  # [p, t, g, c]
    xt = np.empty((128, NT, BCOL), np.float32)
    xt[:, :, : NG * C] = arr.reshape(128, NT, NG * C)
    xt[:, :, NG * C] = 1.0
    xt8 = np.ascontiguousarray(xt.reshape(128, XTW).astype(NPF8))
    return xt8, z8


def _run(logit, label_lst, trace=False):
    nc = _build()
    X = np.asarray(logit, dtype=np.float32).reshape(B, C, HW)
    Y = np.asarray(label_lst).reshape(B, C, HW)[:, :KNOWN].astype(np.float32)

    in_maps = []
    for k in range(NCORES):
        b, half = k // 2, k % 2
        sl = slice(half * HWH, (half + 1) * HWH)
        xt8, z8 = _pack_core(X[b, :, sl], Y[b, :, sl])
        in_maps.append({"xt": xt8, "z": z8})
    return run_bass_kernel_spmd(nc, in_maps, list(range(NCORES)), trace=trace)


def _combine(results, sum_y):
    G = np.zeros((B, C, C), dtype=np.float64)
    sum_x = np.zeros((B, C), dtype=np.float64)
    sabs = np.zeros((B, KNOWN), dtype=np.float64)
    bce_r = np.zeros((B, KNOWN), dtype=np.float64)

    for k in range(NCORES):
        b = k // 2
        r = results[k]
        g = r["g_out"].astype(np.float64)
        st = r["st_out"].astype(np.float64)
        for gi in range(NG):
            slg = slice(gi * C, gi * C + C)
            G[b] += g[slg, slg]
            sum_x[b] += g[slg, NG * C]
        ss = st[:, :NZC].sum(axis=1).reshape(8, KNOWN).sum(axis=0)
        bb = st[:, NZC:].sum(axis=1).reshape(8, KNOWN).sum(axis=0)
        sabs[b] += ss
        bce_r[b] += bb

    num = 0.5 * (sabs + sum_x[:, :KNOWN] + sum_y - HW)
    s = np.einsum("bii->bi", G)

    numk = num + SMOOTH
    denk = s[:, :KNOWN] + sum_y + SMOOTH
    dice = np.mean(1.0 - numk / denk, axis=0)
    bce = -bce_r.sum(axis=0) / (B * HW)
    loss1 = (dice + bce).sum() / KNOWN

    m = sum_x[:, KNOWN:].sum(axis=0) / (B * HW)
    loss2 = np.sum(-np.log(np.clip(m * 50.0, 1e-300, 1.0))) / (C - KNOWN)

    ratio = (G + SMOOTH) / (s[:, :, None] + s[:, None, :] + SMOOTH)
    M = ratio.mean(axis=0)
    loss3 = (M.sum() - np.trace(M)) / (C * (C - 1))

    loss = (loss1 + loss2 + loss3) * 0.1
    f = np.float32
    return f(loss), f(loss1), f(loss2), f(loss3)


def kernel(logit, label_lst, class_lst=None, **_):
    sum_y = (
        np.asarray(label_lst)
        .reshape(B, C, HW)[:, :KNOWN]
        .sum(axis=2, dtype=np.int64)
    )
    res = _run(logit, label_lst, trace=bool(os.environ.get("CDICE_TRACE")))
    out = _combine(res.results, sum_y)
    if os.environ.get("CDICE_TRACE"):
        kernel.last_result = res
    return out



# revision 4
# speedup vs baseline: 1.2783x; 1.2783x over previous
"""CDiceLoss Trainium2 kernel, v3.

Shards B*HW over 8 cores (core = one (batch, half-of-HW) slice). Host packs
ONE fp8(e4m3) slab per core, laid out as 86 "double blocks" of shape
[128 pos, 2 k-subtiles, 217 cols] where the 217 columns are

    [ x (6 groups x 20 ch) | ones | z (6 groups x 16 ch) ]   z = |x+y-1|

One DoubleRow fp8 matmul per double block (lhsT = x|ones cols, rhs = all
217) accumulates in PSUM a [121, 217] result holding: the 6 diagonal 20x20
gram blocks (loss3 + dice denominators), sum_x per (g,ch) and sum_z per
(g,ch) via the ones row (loss2, dice numerators via the |x+y-1| identity).

BCE = sum ln z is computed elementwise: a third of the z columns go straight
through ACT Ln with accumulation; the rest go through a DVE pairwise-product
tree (fp8 mult -> bf16, bf16 mult) that quarters the element count before
ACT Ln. Host combines the tiny per-core stats into (loss, l1, l2, l3).
"""

import os
from contextlib import ExitStack

import numpy as np
import ml_dtypes

import concourse.bass as bass
import concourse.bacc as bacc
import concourse.tile as tile
from concourse import mybir
from concourse.bass_utils import run_bass_kernel_spmd

# ---------------- problem geometry (hardcoded) ----------------
B, C, H, W = 4, 20, 512, 512
HW = H * W                  # 262144
KNOWN = 16
SMOOTH = 1.0
NCORES = 8
HWH = HW // 2               # 131072 positions per core
NG = 6                      # channel-groups per gram block
NDB = 86                    # double blocks: 6*86*256 = HWH + 1024
LG = NDB * 256              # 22016 positions per group
PAD = NG * LG - HWH         # 1024 padded positions (tail of group 5)
XCOLS = NG * C              # 120
ONES = XCOLS                # col 120 = ones
ZOFF = 128                  # z cols start at 128 (cols 121-127 are zero pad;
                            # DoubleRow ISA requires AP step % 16 == 0)
NZCOL = NG * KNOWN          # 96 z cols
BCOL = ZOFF + NZCOL         # 224 cols per k-subtile
DBW = 2 * BCOL              # 448 elements per double block per partition
SLABW = NDB * DBW           # 38528

A_DIR = 32                  # z cols sent straight to ACT Ln
T_TREE = NZCOL - A_DIR      # 64 z cols through the DVE product tree
TH = T_TREE // 2            # 32

CHUNKS = [11, 11, 11, 11, 11, 11, 11, 9]     # double blocks per DMA chunk
assert sum(CHUNKS) == NDB
SUPER = [(0, 33), (33, 66), (66, 86)]        # block ranges per ACT super-chunk
SUPER_AFTER_CHUNK = {2: 0, 5: 1, 7: 2}       # emit super-chunk k after chunk i

FP32 = mybir.dt.float32
BF16 = mybir.dt.bfloat16
F8 = mybir.dt.float8e4
NPF8 = ml_dtypes.float8_e4m3
OP = mybir.AluOpType
AF = mybir.ActivationFunctionType
PM = mybir.MatmulPerfMode

_CACHE = {}


def _build():
    if "nc" in _CACHE:
        return _CACHE["nc"]

    nc = bacc.Bacc(
        "TRN2", target_bir_lowering=False, debug=False, num_devices=NCORES
    )

    slab_d = nc.dram_tensor("slab", [128, SLABW], F8, kind="ExternalInput").ap()
    g_d = nc.dram_tensor("g_out", [XCOLS + 1, BCOL], FP32, kind="ExternalOutput").ap()
    st_d = nc.dram_tensor("st_out", [128, 8], FP32, kind="ExternalOutput").ap()

    with tile.TileContext(nc) as tc, ExitStack() as ctx:
        sing = ctx.enter_context(tc.tile_pool(name="sing", bufs=1))
        lnpool = ctx.enter_context(tc.tile_pool(name="lnpool", bufs=2))
        gp_pool = ctx.enter_context(tc.tile_pool(name="gp", bufs=1, space="PSUM"))

        slab = sing.tile([128, NDB, 2, BCOL], F8)
        r1all = sing.tile([128, NDB, 2, TH], BF16)
        r2all = sing.tile([128, NDB, TH], BF16)
        stats = sing.tile([128, 8], FP32)
        g_ps = gp_pool.tile([128, BCOL], FP32)

        # Ln ACT table preload at t=0, off the first chunk's critical path.
        tdum = sing.tile([1, 8], BF16)
        nc.vector.memset(tdum[:, :], 0.5)
        tdum2 = sing.tile([1, 8], BF16)
        nc.scalar.activation(out=tdum2[:, :], in_=tdum[:, :], func=AF.Ln)

        # Input DMAs, alternating issue queues so descriptor generation and
        # transfer hand off without bubbles.
        qs = [nc.sync, nc.gpsimd]
        c0 = 0
        bounds = []
        for i, nb in enumerate(CHUNKS):
            c1 = c0 + nb
            bounds.append((c0, c1))
            qs[i % 2].dma_start(
                out=slab[:, c0:c1, :, :],
                in_=slab_d[:, c0 * DBW : c1 * DBW],
            )
            c0 = c1

        for i, (c0, c1) in enumerate(bounds):
            # gram + sums: one DoubleRow fp8 matmul per double block
            for j in range(c0, c1):
                nc.tensor.matmul(
                    out=g_ps[:, :],
                    lhsT=slab[:, j, :, 0:ZOFF],
                    rhs=slab[:, j, :, :],
                    start=(j == 0),
                    stop=(j == NDB - 1),
                    perf_mode=PM.DoubleRow,
                    skip_group_check=True,
                )
            # product tree round 1: fp8 pairs -> bf16
            nc.vector.tensor_tensor(
                out=r1all[:, c0:c1, :, :],
                in0=slab[:, c0:c1, :, ZOFF + A_DIR : ZOFF + A_DIR + TH],
                in1=slab[:, c0:c1, :, ZOFF + A_DIR + TH : BCOL],
                op=OP.mult,
            )
            # round 2: bf16 pairs across the 2 k-subtiles (2x mode)
            nc.vector.tensor_tensor(
                out=r2all[:, c0:c1, :],
                in0=r1all[:, c0:c1, 0, :],
                in1=r1all[:, c0:c1, 1, :],
                op=OP.mult,
            )
            sc = SUPER_AFTER_CHUNK.get(i)
            if sc is not None:
                s0, s1 = SUPER[sc]
                nsb = s1 - s0
                # direct ACT Ln (+accumulate) on the first A_DIR z cols
                lnt = lnpool.tile([128, 33, 2, A_DIR], BF16, tag="lnd")
                nc.scalar.activation(
                    out=lnt[:, 0:nsb, :, :],
                    in_=slab[:, s0:s1, :, ZOFF : ZOFF + A_DIR],
                    func=AF.Ln,
                    accum_out=stats[:, sc : sc + 1],
                )
                # ACT Ln (+accumulate) on the tree output
                lnt2 = lnpool.tile([128, 33, TH], BF16, tag="lnt")
                nc.scalar.activation(
                    out=lnt2[:, 0:nsb, :],
                    in_=r2all[:, s0:s1, :],
                    func=AF.Ln,
                    accum_out=stats[:, 4 + sc : 5 + sc],
                )

        g_sb = sing.tile([128, BCOL], FP32)
        nc.vector.tensor_copy(out=g_sb[0:XCOLS + 1, :], in_=g_ps[0:XCOLS + 1, :])
        nc.sync.dma_start(out=g_d, in_=g_sb[0:XCOLS + 1, :])
        nc.gpsimd.dma_start(out=st_d, in_=stats[:, :])

    nc.compile()
    _CACHE["nc"] = nc
    return nc


def _pack_core(Xc, Yc):
    """Xc [20, HWH] f32, Yc [16, HWH] f32 -> slab fp8 [128, SLABW]."""
    Zc = np.abs(Xc[:KNOWN] + Yc - 1.0)

    xp = np.ones((C, NG * LG), np.float32)
    xp[:, :HWH] = Xc
    xp[:, HWH:] = 0.0
    zp = np.ones((KNOWN, NG * LG), np.float32)
    zp[:, :HWH] = Zc
    # [c, g, T, s, p] -> [p, T, s, g, c]
    xa = xp.reshape(C, NG, NDB, 2, 128).transpose(4, 2, 3, 1, 0)
    za = zp.reshape(KNOWN, NG, NDB, 2, 128).transpose(4, 2, 3, 1, 0)

    slab = np.zeros((128, NDB, 2, BCOL), np.float32)
    slab[:, :, :, :XCOLS] = xa.reshape(128, NDB, 2, XCOLS)
    slab[:, :, :, XCOLS] = 1.0
    slab[:, :, :, ZOFF:] = za.reshape(128, NDB, 2, NZCOL)
    return np.ascontiguousarray(slab.reshape(128, SLABW).astype(NPF8))


def _run(logit, label_lst, trace=False):
    nc = _build()
    X = np.asarray(logit, dtype=np.float32).reshape(B, C, HW)
    Y = np.asarray(label_lst).reshape(B, C, HW)[:, :KNOWN].astype(np.float32)

    in_maps = []
    for k in range(NCORES):
        b, half = k // 2, k % 2
        sl = slice(half * HWH, (half + 1) * HWH)
        in_maps.append({"slab": _pack_core(X[b, :, sl], Y[b, :, sl])})
    return run_bass_kernel_spmd(nc, in_maps, list(range(NCORES)), trace=trace)


def _combine(results, sum_y):
    G = np.zeros((B, C, C), dtype=np.float64)
    sum_x = np.zeros((B, C), dtype=np.float64)
    sum_z = np.zeros((B, KNOWN), dtype=np.float64)
    bce_total = 0.0

    for k in range(NCORES):
        b = k // 2
        g = results[k]["g_out"].astype(np.float64)
        st = results[k]["st_out"].astype(np.float64)
        for gi in range(NG):
            slg = slice(gi * C, gi * C + C)
            G[b] += g[slg, slg]
            sum_x[b] += g[XCOLS, slg]
            sum_z[b] += g[XCOLS, ZOFF + gi * KNOWN : ZOFF + (gi + 1) * KNOWN]
        sum_z[b] -= PAD  # z pad value is 1.0
        # tree lns are of 4-way products: each covers 4 z values
        bce_total += st[:, 0:4].sum() + st[:, 4:8].sum()

    # sum |x+y-1| identity: sum(xy) = (sum_z + sum_x + sum_y - N) / 2
    num = 0.5 * (sum_z + sum_x[:, :KNOWN] + sum_y - HW)
    s = np.einsum("bii->bi", G)

    numk = num + SMOOTH
    denk = s[:, :KNOWN] + sum_y + SMOOTH
    dice = np.mean(1.0 - numk / denk, axis=0)
    bce_c_total = -bce_total / (B * HW)   # = sum_c bce_c
    loss1 = (dice.sum() + bce_c_total) / KNOWN

    m = sum_x[:, KNOWN:].sum(axis=0) / (B * HW)
    loss2 = np.sum(-np.log(np.clip(m * 50.0, 1e-300, 1.0))) / (C - KNOWN)

    ratio = (G + SMOOTH) / (s[:, :, None] + s[:, None, :] + SMOOTH)
    M = ratio.mean(axis=0)
    loss3 = (M.sum() - np.trace(M)) / (C * (C - 1))

    loss = (loss1 + loss2 + loss3) * 0.1
    f = np.float32
    return f(loss), f(loss1), f(loss2), f(loss3)


def kernel(logit, label_lst, class_lst=None, **_):
    sum_y = (
        np.asarray(label_lst)
        .reshape(B, C, HW)[:, :KNOWN]
        .sum(axis=2, dtype=np.int64)
    )
    res = _run(logit, label_lst, trace=bool(os.environ.get("CDICE_TRACE")))
    out = _combine(res.results, sum_y)
    if os.environ.get("CDICE_TRACE"):
        kernel.last_result = res
    return out
